# revision 3
# baseline (speedup 1.0000x reference)
"""ColBERT MaxSim loss kernel for Trainium2 (8 NeuronCores, SPMD).

Strategy: shard documents across the 8 cores (32 docs each); queries are
replicated. Each core projects + L2-normalizes its doc tokens and all query
tokens (PE transpose -> matmul over H=768 -> normalize -> PE transpose back),
computes the MaxSim interaction with D=64-contraction matmuls, reduces
max-over-Ld on VectorE straight out of PSUM, and accumulates the Lq-sum with a
block-diagonal ones matmul in PSUM. Each core emits a [32 queries x 32 docs]
score block; the host concatenates the 8 blocks into the full [32, 256] score
matrix and finishes with the (tiny) cross-entropy reduction.
"""

import sys

import numpy as np

try:
    import concourse.bass as bass
except ImportError:  # pragma: no cover - fallback for bare environments
    sys.path.insert(0, "/opt/trn_rl_repo")
    import concourse.bass as bass

import concourse.mybir as mybir
import concourse.tile as tile
from concourse.bass_utils import run_bass_kernel_spmd
from concourse.masks import make_identity

F32 = mybir.dt.float32
F32R = mybir.dt.float32r

# Problem shape (hardcoded; see module docstring).
BQ, LQ, BD, LD, H, D = 32, 32, 256, 180, 768, 64
NCORES = 8
BD_LOC = BD // NCORES  # 32 docs per core
TD = BD_LOC * LD  # 5760 doc tokens per core
TQ = BQ * LQ  # 1024 query tokens
KT = H // 128  # 6 contraction k-tiles
NB_D = TD // 128  # 45 doc token blocks
NB_Q = TQ // 128  # 8 query token blocks
Q_PER_BLOCK = 128 // LQ  # 4 queries per 128-token block
SIM_CHUNK = 4  # docs per sim chunk (2 pair-matmuls of N=360)
N_CHUNKS = BD_LOC // SIM_CHUNK  # 8
EPS = 1e-12

# Perf knobs (validated against the jax reference on hardware):
# fp32r runs the PE at 1 cycle/row instead of fp32's 4 for the big moving
# operands; the transposes are pure data movement.
SIM_DT = F32  # dtype for the MaxSim matmul operands
TRANS_DT = F32  # dtype for PE transposes


def _mm_cast(ap, dt):
    return ap.bitcast(dt) if dt != F32 else ap


def _t_cast(ap, dt):
    return ap.bitcast(dt) if dt != F32 else ap


def _process_block(nc, pools, wt_sb, identity, src, b, out_t, copy_parity):
    """Load one 128-token block, transpose, project, normalize, deposit into
    out_t[:, b*128:(b+1)*128] (the [64, tokens] projected+normalized layout)."""
    dload, dtos, dn, small, ps_t, ps_pd = pools
    ident_t = _t_cast(identity, TRANS_DT)

    nat = dload.tile([128, H], F32, tag="nat")
    nc.sync.dma_start(out=nat, in_=src[b * 128 : (b + 1) * 128, :])

    tsb = dtos.tile([128, KT, 128], F32, tag="tsb")
    for g in range(2):
        pst = ps_t.tile([128, 3, 128], F32, tag="pst")
        for j in range(3):
            k = g * 3 + j
            nc.tensor.transpose(
                pst[:, j, :],
                _t_cast(nat[:, k * 128 : (k + 1) * 128], TRANS_DT),
                ident_t,
            )
        if (copy_parity + g) % 2 == 0:
            nc.vector.tensor_copy(out=tsb[:, g * 3 : g * 3 + 3, :], in_=pst)
        else:
            nc.scalar.copy(out=tsb[:, g * 3 : g * 3 + 3, :], in_=pst)

    # Project: d[tok, 64] accumulated over 6 k-tiles.
    pd = ps_pd.tile([128, D], F32, tag="pd")
    for k in range(KT):
        nc.tensor.matmul(
            pd,
            lhsT=tsb[:, k, :],
            rhs=wt_sb[:, k, :],
            start=(k == 0),
            stop=(k == KT - 1),
        )

    # L2 normalize per token (rows): 1/max(sqrt(sum(d^2)), eps).
    sq_scratch = dn.tile([128, D], F32, tag="sqs")
    ssq = small.tile([128, 1], F32, tag="ssq")
    nc.scalar.activation(
        out=sq_scratch,
        in_=pd,
        func=mybir.ActivationFunctionType.Square,
        accum_out=ssq,
    )
    nrm = small.tile([128, 1], F32, tag="nrm")
    nc.scalar.activation(out=nrm, in_=ssq, func=mybir.ActivationFunctionType.Sqrt)
    nc.vector.tensor_scalar_max(out=nrm, in0=nrm, scalar1=EPS)
    rn = small.tile([128, 1], F32, tag="rn")
    nc.vector.reciprocal(out=rn, in_=nrm)
    dnrm = dn.tile([128, D], F32, tag="dnrm")
    nc.vector.tensor_scalar_mul(out=dnrm, in0=pd, scalar1=rn)

    # Transpose [128 tok, 64] -> [64, 128 tok] and deposit.
    ptr = ps_pd.tile([64, 128], F32, tag="pd")
    nc.tensor.transpose(ptr, _t_cast(dnrm, TRANS_DT), ident_t)
    if copy_parity % 2 == 0:
        nc.scalar.copy(out=out_t[:, b * 128 : (b + 1) * 128], in_=ptr)
    else:
        nc.vector.tensor_copy(out=out_t[:, b * 128 : (b + 1) * 128], in_=ptr)


def _emit_sim_chunk(nc, ps_s, qt, dt_, maxsim_all, c):
    """MaxSim for docs [c*SIM_CHUNK, (c+1)*SIM_CHUNK) against all query blocks."""
    col0 = c * SIM_CHUNK * LD
    for qb in range(NB_Q):
        ps = ps_s.tile([128, 2, 512], F32, tag="sim")
        for j in range(2):
            nc.tensor.matmul(
                ps[:, j, 0:360],
                lhsT=_mm_cast(qt[:, qb * 128 : (qb + 1) * 128], SIM_DT),
                rhs=_mm_cast(dt_[:, col0 + j * 360 : col0 + (j + 1) * 360], SIM_DT),
                start=True,
                stop=True,
            )
        in_view = ps[:, :, 0:360].rearrange("p j (d l) -> p j d l", d=2)
        out_view = maxsim_all[
            :, qb, c * SIM_CHUNK : (c + 1) * SIM_CHUNK
        ].rearrange("p (j d) -> p j d", j=2)
        nc.vector.reduce_max(out=out_view, in_=in_view, axis=mybir.AxisListType.X)


def _kernel_body(tc, doc, qry, wt, qmask, scores_out):
    nc = tc.nc
    with (
        tc.tile_pool(name="const", bufs=1) as const,
        tc.tile_pool(name="dload", bufs=3) as dload,
        tc.tile_pool(name="dtos", bufs=3) as dtos,
        tc.tile_pool(name="dn", bufs=3) as dn,
        tc.tile_pool(name="small", bufs=6) as small,
        tc.tile_pool(name="ps_t", bufs=2, space="PSUM") as ps_t,
        tc.tile_pool(name="ps_pd", bufs=2, space="PSUM") as ps_pd,
        tc.tile_pool(name="ps_s", bufs=2, space="PSUM") as ps_s,
    ):
        identity = const.tile([128, 128], F32)
        make_identity(nc, identity)

        # W.T as 6 k-tiles: wt_sb[p, k, d] = W.T[k*128+p, d]
        wt_sb = const.tile([128, KT, D], F32)
        nc.sync.dma_start(
            out=wt_sb, in_=wt[:, :].rearrange("(k p) d -> p k d", p=128)
        )
        qmask_sb = const.tile([128, NB_Q, BQ], F32)
        nc.sync.dma_start(out=qmask_sb, in_=qmask[:, :, :])

        qt = const.tile([64, TQ], F32)  # normalized projected queries, [64, tok]
        dt_ = const.tile([64, TD], F32)  # normalized projected docs, [64, tok]
        maxsim_all = const.tile([128, NB_Q, BD_LOC], F32)

        pools = (dload, dtos, dn, small, ps_t, ps_pd)

        for b in range(NB_Q):
            _process_block(nc, pools, wt_sb, identity, qry, b, qt, b)

        next_chunk = 0
        for b in range(NB_D):
            _process_block(nc, pools, wt_sb, identity, doc, b, dt_, NB_Q + b)
            done_tokens = (b + 1) * 128
            while (
                next_chunk < N_CHUNKS
                and (next_chunk + 1) * SIM_CHUNK * LD <= done_tokens
            ):
                _emit_sim_chunk(nc, ps_s, qt, dt_, maxsim_all, next_chunk)
                next_chunk += 1
        while next_chunk < N_CHUNKS:
            _emit_sim_chunk(nc, ps_s, qt, dt_, maxsim_all, next_chunk)
            next_chunk += 1

        # Lq-sum via block-diagonal ones: scores[q, d] = sum_i maxsim[q*32+i, d]
        scores_ps = ps_s.tile([BQ, BD_LOC], F32, tag="sim")
        for qb in range(NB_Q):
            nc.tensor.matmul(
                scores_ps,
                lhsT=qmask_sb[:, qb, :],
                rhs=maxsim_all[:, qb, :],
                start=(qb == 0),
                stop=(qb == NB_Q - 1),
            )
        scores_sb = small.tile([BQ, BD_LOC], F32, tag="scores")
        nc.vector.tensor_copy(out=scores_sb, in_=scores_ps)
        nc.sync.dma_start(out=scores_out[:, :], in_=scores_sb)


def split_multi_waits(nc, max_waits=1):
    """The public neuronxcc walrus only encodes one inline sync-wait per
    instruction; Tile's scheduler attaches several. Split the excess into
    preceding same-engine nop-waits (engine queues execute in order, so the
    semantics are identical)."""
    for f in nc.m.functions:
        for blk in f.blocks:
            new_insts = []
            for inst in blk.instructions:
                si = inst.sync_info
                if si is not None and len(si.on_wait) > max_waits:
                    waits = list(si.on_wait)
                    for w in waits[:-max_waits]:
                        new_insts.append(
                            mybir.InstNoOp(
                                name=nc.get_next_instruction_name(),
                                ins=[],
                                outs=[],
                                engine=inst.engine,
                                sync_info=mybir.SyncInfo(on_wait=[w], on_update=[]),
                            )
                        )
                    inst.sync_info = mybir.SyncInfo(
                        on_wait=waits[-max_waits:], on_update=list(si.on_update)
                    )
                new_insts.append(inst)
            blk.instructions = new_insts
    return nc


def build_bass():
    nc = bass.Bass()
    doc = nc.dram_tensor("doc", [TD, H], F32, kind="ExternalInput")
    qry = nc.dram_tensor("qry", [TQ, H], F32, kind="ExternalInput")
    wt = nc.dram_tensor("wt", [H, D], F32, kind="ExternalInput")
    qmask = nc.dram_tensor("qmask", [128, NB_Q, BQ], F32, kind="ExternalInput")
    scores_out = nc.dram_tensor("scores", [BQ, BD_LOC], F32, kind="ExternalOutput")
    with tile.TileContext(nc) as tc:
        _kernel_body(tc, doc, qry, wt, qmask, scores_out)
    split_multi_waits(nc)
    return nc


def _build_qmask():
    qmask = np.zeros((128, NB_Q, BQ), dtype=np.float32)
    p = np.arange(128)
    for qb in range(NB_Q):
        qmask[p, qb, qb * Q_PER_BLOCK + p // LQ] = 1.0
    return qmask


_NC_CACHE = None


def _get_nc():
    global _NC_CACHE
    if _NC_CACHE is None:
        _NC_CACHE = build_bass()
    return _NC_CACHE


def _make_in_maps(qry_emb, doc_emb, W):
    wt = np.ascontiguousarray(W.T.astype(np.float32))  # [768, 64]
    qry = np.ascontiguousarray(qry_emb.reshape(TQ, H).astype(np.float32))
    qmask = _build_qmask()
    in_maps = []
    for c in range(NCORES):
        docs = np.ascontiguousarray(
            doc_emb[c * BD_LOC : (c + 1) * BD_LOC].reshape(TD, H).astype(np.float32)
        )
        in_maps.append({"doc": docs, "qry": qry, "wt": wt, "qmask": qmask})
    return in_maps


def _finish_loss(score_blocks, group_size):
    scores = np.concatenate(score_blocks, axis=1).astype(np.float64)  # [32, 256]
    labels = np.arange(BQ) * int(group_size)
    m = scores.max(axis=1, keepdims=True)
    lse = m[:, 0] + np.log(np.exp(scores - m).sum(axis=1))
    loss = np.mean(lse - scores[np.arange(BQ), labels])
    return np.float32(loss)


def kernel(qry_emb, doc_emb, W, group_size, _trace=False):
    nc = _get_nc()
    in_maps = _make_in_maps(np.asarray(qry_emb), np.asarray(doc_emb), np.asarray(W))
    res = run_bass_kernel_spmd(nc, in_maps, list(range(NCORES)), trace=_trace)
    blocks = [res.results[c]["scores"] for c in range(NCORES)]
    loss = _finish_loss(blocks, group_size)
    if _trace:
        return loss, res
    return loss


# revision 8
# speedup vs baseline: 57.8124x; 57.8124x over previous
"""ColBERT MaxSim loss kernel for Trainium2 (8 NeuronCores, SPMD).

Strategy: shard documents across the 8 cores (32 docs each); queries are
replicated. Each core projects + L2-normalizes its doc tokens and all query
tokens (PE transpose -> matmul over H=768 -> normalize -> PE transpose back),
computes the MaxSim interaction with D=64-contraction matmuls, reduces
max-over-Ld on VectorE straight out of PSUM, and accumulates the Lq-sum with a
block-diagonal ones matmul in PSUM. Each core emits a [32 queries x 32 docs]
score block; the host concatenates the 8 blocks into the full [32, 256] score
matrix and finishes with the (tiny) cross-entropy reduction.
"""

import sys

import numpy as np

try:
    import concourse.bass as bass
except ImportError:  # pragma: no cover - fallback for bare environments
    sys.path.insert(0, "/opt/trn_rl_repo")
    import concourse.bass as bass

import concourse.mybir as mybir
import concourse.tile as tile
from concourse.bass_utils import run_bass_kernel_spmd
from concourse.masks import make_identity

F32 = mybir.dt.float32
F32R = mybir.dt.float32r

# Problem shape (hardcoded; see module docstring).
BQ, LQ, BD, LD, H, D = 32, 32, 256, 180, 768, 64
NCORES = 8
BD_LOC = BD // NCORES  # 32 docs per core
TD = BD_LOC * LD  # 5760 doc tokens per core
TQ = BQ * LQ  # 1024 query tokens
KT = H // 128  # 6 contraction k-tiles
NB_D = TD // 128  # 45 doc token blocks
NB_Q = TQ // 128  # 8 query token blocks
Q_PER_BLOCK = 128 // LQ  # 4 queries per 128-token block
SIM_CHUNK = 4  # docs per sim chunk (2 pair-matmuls of N=360)
N_CHUNKS = BD_LOC // SIM_CHUNK  # 8
EPS = 1e-12

# Perf knobs (validated against the jax reference on hardware):
# fp32r runs the PE at 1 cycle/row instead of fp32's 4 for the big moving
# operands; the transposes are pure data movement.
SIM_DT = F32  # dtype for the MaxSim matmul operands
TRANS_DT = F32  # dtype for PE transposes


def _mm_cast(ap, dt):
    return ap.bitcast(dt) if dt != F32 else ap


def _t_cast(ap, dt):
    return ap.bitcast(dt) if dt != F32 else ap


def _process_block(nc, pools, wt_sb, identity, src, b, out_t, copy_parity):
    """Load one 128-token block, transpose, project, normalize, deposit into
    out_t[:, b*128:(b+1)*128] (the [64, tokens] projected+normalized layout)."""
    dload, dtos, dn, small, ps_t, ps_pd = pools
    ident_t = _t_cast(identity, TRANS_DT)

    nat = dload.tile([128, H], F32, tag="nat")
    nc.sync.dma_start(out=nat, in_=src[b * 128 : (b + 1) * 128, :])

    tsb = dtos.tile([128, KT, 128], F32, tag="tsb")
    for g in range(2):
        pst = ps_t.tile([128, 3, 128], F32, tag="pst")
        for j in range(3):
            k = g * 3 + j
            nc.tensor.transpose(
                pst[:, j, :],
                _t_cast(nat[:, k * 128 : (k + 1) * 128], TRANS_DT),
                ident_t,
            )
        if (copy_parity + g) % 2 == 0:
            nc.vector.tensor_copy(out=tsb[:, g * 3 : g * 3 + 3, :], in_=pst)
        else:
            nc.scalar.copy(out=tsb[:, g * 3 : g * 3 + 3, :], in_=pst)

    # Project: d[tok, 64] accumulated over 6 k-tiles.
    pd = ps_pd.tile([128, D], F32, tag="pd")
    for k in range(KT):
        nc.tensor.matmul(
            pd,
            lhsT=tsb[:, k, :],
            rhs=wt_sb[:, k, :],
            start=(k == 0),
            stop=(k == KT - 1),
        )

    # L2 normalize per token (rows): 1/max(sqrt(sum(d^2)), eps).
    sq_scratch = dn.tile([128, D], F32, tag="sqs")
    ssq = small.tile([128, 1], F32, tag="ssq")
    nc.scalar.activation(
        out=sq_scratch,
        in_=pd,
        func=mybir.ActivationFunctionType.Square,
        accum_out=ssq,
    )
    nrm = small.tile([128, 1], F32, tag="nrm")
    nc.scalar.activation(out=nrm, in_=ssq, func=mybir.ActivationFunctionType.Sqrt)
    nc.vector.tensor_scalar_max(out=nrm, in0=nrm, scalar1=EPS)
    rn = small.tile([128, 1], F32, tag="rn")
    nc.vector.reciprocal(out=rn, in_=nrm)
    dnrm = dn.tile([128, D], F32, tag="dnrm")
    nc.vector.tensor_scalar_mul(out=dnrm, in0=pd, scalar1=rn)

    # Transpose [128 tok, 64] -> [64, 128 tok] and deposit.
    ptr = ps_pd.tile([64, 128], F32, tag="pd")
    nc.tensor.transpose(ptr, _t_cast(dnrm, TRANS_DT), ident_t)
    if copy_parity % 2 == 0:
        nc.scalar.copy(out=out_t[:, b * 128 : (b + 1) * 128], in_=ptr)
    else:
        nc.vector.tensor_copy(out=out_t[:, b * 128 : (b + 1) * 128], in_=ptr)


def _emit_sim_chunk(nc, ps_s, qt, dt_, maxsim_all, c):
    """MaxSim for docs [c*SIM_CHUNK, (c+1)*SIM_CHUNK) against all query blocks."""
    col0 = c * SIM_CHUNK * LD
    for qb in range(NB_Q):
        ps = ps_s.tile([128, 2, 512], F32, tag="sim")
        for j in range(2):
            nc.tensor.matmul(
                ps[:, j, 0:360],
                lhsT=_mm_cast(qt[:, qb * 128 : (qb + 1) * 128], SIM_DT),
                rhs=_mm_cast(dt_[:, col0 + j * 360 : col0 + (j + 1) * 360], SIM_DT),
                start=True,
                stop=True,
            )
        in_view = ps[:, :, 0:360].rearrange("p j (d l) -> p j d l", d=2)
        out_view = maxsim_all[
            :, qb, c * SIM_CHUNK : (c + 1) * SIM_CHUNK
        ].rearrange("p (j d) -> p j d", j=2)
        nc.vector.reduce_max(out=out_view, in_=in_view, axis=mybir.AxisListType.X)


def _kernel_body(tc, doc, qry, wt, qmask, scores_out, repeat=1):
    nc = tc.nc
    with (
        tc.tile_pool(name="const", bufs=1) as const,
        tc.tile_pool(name="dload", bufs=3) as dload,
        tc.tile_pool(name="dtos", bufs=3) as dtos,
        tc.tile_pool(name="dn", bufs=3) as dn,
        tc.tile_pool(name="small", bufs=6) as small,
        tc.tile_pool(name="ps_t", bufs=2, space="PSUM") as ps_t,
        tc.tile_pool(name="ps_pd", bufs=2, space="PSUM") as ps_pd,
        tc.tile_pool(name="ps_s", bufs=2, space="PSUM") as ps_s,
    ):
        identity = const.tile([128, 128], F32)
        make_identity(nc, identity)

        # W.T as 6 k-tiles: wt_sb[p, k, d] = W.T[k*128+p, d]
        wt_sb = const.tile([128, KT, D], F32)
        nc.sync.dma_start(
            out=wt_sb, in_=wt[:, :].rearrange("(k p) d -> p k d", p=128)
        )
        qmask_sb = const.tile([128, NB_Q, BQ], F32)
        nc.sync.dma_start(out=qmask_sb, in_=qmask[:, :, :])

        qt = const.tile([64, TQ], F32)  # normalized projected queries, [64, tok]
        dt_ = const.tile([64, TD], F32)  # normalized projected docs, [64, tok]
        maxsim_all = const.tile([128, NB_Q, BD_LOC], F32)

        pools = (dload, dtos, dn, small, ps_t, ps_pd)

        def _one_pass():
            for b in range(NB_Q):
                _process_block(nc, pools, wt_sb, identity, qry, b, qt, b)

            next_chunk = 0
            for b in range(NB_D):
                _process_block(nc, pools, wt_sb, identity, doc, b, dt_, NB_Q + b)
                done_tokens = (b + 1) * 128
                while (
                    next_chunk < N_CHUNKS
                    and (next_chunk + 1) * SIM_CHUNK * LD <= done_tokens
                ):
                    _emit_sim_chunk(nc, ps_s, qt, dt_, maxsim_all, next_chunk)
                    next_chunk += 1
            while next_chunk < N_CHUNKS:
                _emit_sim_chunk(nc, ps_s, qt, dt_, maxsim_all, next_chunk)
                next_chunk += 1

            # Lq-sum via block-diag ones: scores[q, d] = sum_i maxsim[q*32+i, d]
            scores_ps = ps_s.tile([BQ, BD_LOC], F32, tag="sim")
            for qb in range(NB_Q):
                nc.tensor.matmul(
                    scores_ps,
                    lhsT=qmask_sb[:, qb, :],
                    rhs=maxsim_all[:, qb, :],
                    start=(qb == 0),
                    stop=(qb == NB_Q - 1),
                )
            scores_sb = small.tile([BQ, BD_LOC], F32, tag="scores")
            nc.vector.tensor_copy(out=scores_sb, in_=scores_ps)
            nc.sync.dma_start(out=scores_out[:, :], in_=scores_sb)

        if repeat == 1:
            _one_pass()
        else:
            with tc.For_i(0, repeat, 1):
                _one_pass()


def split_multi_waits(nc, max_waits=1):
    """The public neuronxcc walrus only encodes one inline sync-wait per
    instruction; Tile's scheduler attaches several. Split the excess into
    preceding same-engine nop-waits (engine queues execute in order, so the
    semantics are identical)."""
    for f in nc.m.functions:
        for blk in f.blocks:
            new_insts = []
            for inst in blk.instructions:
                si = inst.sync_info
                if si is not None and len(si.on_wait) > max_waits:
                    waits = list(si.on_wait)
                    for w in waits[:-max_waits]:
                        new_insts.append(
                            mybir.InstNoOp(
                                name=nc.get_next_instruction_name(),
                                ins=[],
                                outs=[],
                                engine=inst.engine,
                                sync_info=mybir.SyncInfo(on_wait=[w], on_update=[]),
                            )
                        )
                    inst.sync_info = mybir.SyncInfo(
                        on_wait=waits[-max_waits:], on_update=list(si.on_update)
                    )
                new_insts.append(inst)
            blk.instructions = new_insts
    return nc


def build_bass(repeat=1):
    nc = bass.Bass()
    doc = nc.dram_tensor("doc", [TD, H], F32, kind="ExternalInput")
    qry = nc.dram_tensor("qry", [TQ, H], F32, kind="ExternalInput")
    wt = nc.dram_tensor("wt", [H, D], F32, kind="ExternalInput")
    qmask = nc.dram_tensor("qmask", [128, NB_Q, BQ], F32, kind="ExternalInput")
    scores_out = nc.dram_tensor("scores", [BQ, BD_LOC], F32, kind="ExternalOutput")
    with tile.TileContext(nc) as tc:
        _kernel_body(tc, doc, qry, wt, qmask, scores_out, repeat=repeat)
    split_multi_waits(nc)
    return nc


def _build_qmask():
    qmask = np.zeros((128, NB_Q, BQ), dtype=np.float32)
    p = np.arange(128)
    for qb in range(NB_Q):
        qmask[p, qb, qb * Q_PER_BLOCK + p // LQ] = 1.0
    return qmask


_NC_CACHE = None


def _get_nc():
    global _NC_CACHE
    if _NC_CACHE is None:
        _NC_CACHE = build_bass()
    return _NC_CACHE


def _make_in_maps(qry_emb, doc_emb, W):
    wt = np.ascontiguousarray(W.T.astype(np.float32))  # [768, 64]
    qry = np.ascontiguousarray(qry_emb.reshape(TQ, H).astype(np.float32))
    qmask = _build_qmask()
    in_maps = []
    for c in range(NCORES):
        docs = np.ascontiguousarray(
            doc_emb[c * BD_LOC : (c + 1) * BD_LOC].reshape(TD, H).astype(np.float32)
        )
        in_maps.append({"doc": docs, "qry": qry, "wt": wt, "qmask": qmask})
    return in_maps


def _finish_loss(score_blocks, group_size):
    scores = np.concatenate(score_blocks, axis=1).astype(np.float64)  # [32, 256]
    labels = np.arange(BQ) * int(group_size)
    m = scores.max(axis=1, keepdims=True)
    lse = m[:, 0] + np.log(np.exp(scores - m).sum(axis=1))
    loss = np.mean(lse - scores[np.arange(BQ), labels])
    return np.float32(loss)


def kernel(qry_emb, doc_emb, W, group_size, _trace=False):
    nc = _get_nc()
    in_maps = _make_in_maps(np.asarray(qry_emb), np.asarray(doc_emb), np.asarray(W))
    res = run_bass_kernel_spmd(nc, in_maps, list(range(NCORES)), trace=_trace)
    blocks = [res.results[c]["scores"] for c in range(NCORES)]
    loss = _finish_loss(blocks, group_size)
    if _trace:
        return loss, res
    return loss
